# revision 16
# baseline (speedup 1.0000x reference)
"""Trainium2 Bass kernel for a dense recurrent scan (nn_CXBPU_55611236549128).

Math (per timestep t, K=4 microsteps):
    inj  = x_t @ W_in.T + b_in                  scattered into sensory_indices
    h    = relu(h @ W_rec.T + scatter(inj))     microstep 0
    h    = relu(h @ W_rec.T)                    microsteps 1..K-1
    out_t = h[:, output_indices] @ W_out.T + b_out

Sharding: data-parallel over batch, 8 rows per core, W_rec replicated.

Per-core design (feature-major "hT" layout [128 partitions, 16 chunks x 8 batch]):
  - W_rec.T resident in SBUF as fp16 (single pass; quantization noise averages
    out over the 2048-wide contraction, end-to-end rel err ~8e-4), streamed as
    the *moving* matmul operand every microstep. 4 PE column groups
    (tile_position=(0,32j)) give 4 concurrent 512-wide streams = the PE
    inflow roofline (~216ns per slot group of 4 MMs, 16 groups/microstep).
  - Every pipelined structure is split into per-bank / per-quarter TILES
    (4 psum banks, 4 evac tiles, 4 psumT tiles, 4 hT quarter tiles): Tile
    tracks dependencies at tile granularity, so a single wide tile creates
    false WAR/RAW edges that serialize the whole pipeline (this was the
    dominant cost of earlier versions).
  - Mains are bank-outer so each bank's evacuation (psum -> SBUF fp16, half
    on DVE, half on ACT) spreads across the stream instead of bunching at
    the end.
  - A "transpose-sum" matmul per 128-chunk against a 0/1 selector (i128)
    folds the 4 partition groups back into feature-major hT (exact in fp32
    PSUM). The 16 tMMs are pinned after the mains via add_dep_helper so the
    scheduler cannot head-of-line-block the PE queue with them. relu (DVE)
    per quarter; microstep 0 fuses the injection add (DMA'd dense, host
    precomputed) before the relu.
  - Readout is done ON THE HOST: the final hT of each timestep is staged to
    DRAM on otherwise-idle DMA engines (zero PE cost) and the tiny
    h[:, oidx] @ W_out.T runs in numpy after the kernel.
"""

import os
from contextlib import ExitStack

import numpy as np

N = 2048
B = 64
T = 128
NCORES = 8
BPC = B // NCORES  # 8 batch rows per core
NCHUNK = N // 128  # 16

_CACHE = {}

# 'fp16' = single-pass fp16 (fast), 'fp16x2' = two-pass fp16 split (more exact)
MODE = os.environ.get("KERNEL_MM_MODE", "fp16")


def _build_nc(n_steps, mode=MODE):
    import concourse.bass as bass
    import concourse.mybir as mybir
    import concourse.tile as tile
    from bass_rust import add_dep_helper
    from concourse import bacc

    f32 = mybir.dt.float32
    f16 = mybir.dt.float16
    fmm = f16
    npass = 2 if mode == "fp16x2" else 1
    nc = bacc.Bacc(trn_type="TRN2")

    NSLAB = npass * NCHUNK

    wt_d = nc.dram_tensor("wt", [NSLAB * 128, N], fmm, kind="ExternalInput")
    injd_d = nc.dram_tensor("injd", [n_steps * 128, NCHUNK * BPC], fmm,
                            kind="ExternalInput")
    i128_d = nc.dram_tensor("i128", [128, BPC], fmm, kind="ExternalInput")
    hs_d = nc.dram_tensor("hs", [n_steps * 128, NCHUNK * BPC], fmm,
                          kind="ExternalOutput")

    with tile.TileContext(nc) as tc, ExitStack() as ctx:
        const = ctx.enter_context(tc.tile_pool(name="const", bufs=1))
        hpool = ctx.enter_context(tc.tile_pool(name="h", bufs=2))
        ipool = ctx.enter_context(tc.tile_pool(name="injd", bufs=2))
        epool = ctx.enter_context(tc.tile_pool(name="evac", bufs=2))
        ppool = ctx.enter_context(tc.tile_pool(name="psum", bufs=1, space="PSUM"))
        tpool = ctx.enter_context(tc.tile_pool(name="psumT", bufs=1, space="PSUM"))

        # resident W^T slabs: slab u at cols [u*2048, ...). Spread the load
        # across both HWDGE families + SWDGE.
        wt = const.tile([128, NSLAB * N], fmm)
        for u in range(NSLAB):
            eng = (nc.sync, nc.scalar, nc.gpsimd)[u % 3]
            eng.dma_start(wt[:, u * N : (u + 1) * N], wt_d[u * 128 : (u + 1) * 128, :])
        i128 = const.tile([128, BPC], fmm)
        nc.sync.dma_start(i128[:], i128_d[:])

        # one PSUM tile per bank so evac reads of bank n don't create false
        # WAR edges against matmul writes of other banks (Tile tracks
        # dependencies at tile granularity)
        psum = [ppool.tile([128, 512], f32, name=f"psum{n}") for n in range(4)]
        for n in range(4):
            nc.vector.memset(psum[n][:], 0.0)

        # persistent transpose-sum targets
        psumT = [tpool.tile([128, 4 * BPC], f32, name=f"psumT{q}")
                 for q in range(4)]

        # hT split into 4 quarter tiles (chunks 4q..4q+3) so round-r matmuls
        # only depend on relu(r), not all four relus
        hT = [hpool.tile([128, 4 * BPC], fmm, name=f"hT{q}") for q in range(4)]
        for q in range(4):
            nc.vector.memset(hT[q][:], 0.0)

        tc.strict_bb_all_engine_barrier()

        # Work deferred into the next microstep's stream (previous timestep's
        # readout) so its PE waits land after the producing relu completes.
        pending = []

        for t in range(n_steps):
            injd = ipool.tile([128, NCHUNK * BPC], fmm)
            nc.sync.dma_start(injd[:], injd_d[t * 128 : (t + 1) * 128, :])
            for s in range(4):
                evac = [epool.tile([128, 512], fmm, name=f"evac{n}") for n in range(4)]
                hT_new = [hpool.tile([128, 4 * BPC], fmm, name=f"hTn{q}") for q in range(4)]

                if s == 0:
                    # previous timestep's h staging DMA (for host-side readout)
                    for fn in pending:
                        fn()
                    pending = []

                # ---- transpose-sum + relu for chunk-quarter q (== psum
                # bank q == next microstep's round q). Pinned to an anchor
                # main MM so the scheduler cannot head-of-line-block the PE
                # queue with a tmm whose evacuation is still in flight.
                def tail(q, anchor, s=s, evac=evac, hT_new=hT_new, injd=injd):
                    for c in range(4):
                        mm = nc.tensor.matmul(
                            psumT[q][:, c * BPC : (c + 1) * BPC],
                            lhsT=evac[q][:, c * 128 : (c + 1) * 128],
                            rhs=i128[:],
                            start=True,
                            stop=True,
                        )
                        add_dep_helper(mm.ins, anchor.ins, sync=False,
                                       reason="pin tmm after mains")
                    if s == 0:
                        # injection lands only on microstep 0: relu(psum+inj)
                        nc.vector.tensor_add(
                            hT_new[q][:], psumT[q][:],
                            injd[:, q * 4 * BPC : (q + 1) * 4 * BPC])
                        nc.vector.tensor_relu(hT_new[q][:], hT_new[q][:])
                    else:
                        nc.vector.tensor_relu(hT_new[q][:], psumT[q][:])

                # ---- main rounds, bank-outer (banks complete early so their
                # evacuations spread across the whole stream instead of
                # bunching at the end). Quarters 0/1 are emitted after bank 2
                # (their evacs are long complete, their relus land early on
                # the DVE queue); quarters 2/3 after bank 3, where the
                # scheduler fills the evac-3 wait with the next microstep's
                # first groups.
                last_main = None
                for n in range(4):
                    for p in range(npass):
                        for r in range(4):
                            for j in range(4):
                                kk = 4 * r + j
                                u = p * NCHUNK + kk
                                last_main = nc.tensor.matmul(
                                    psum[n][32 * j : 32 * j + BPC, :],
                                    lhsT=hT[r][:, j * BPC : (j + 1) * BPC],
                                    rhs=wt[:, u * N + 512 * n : u * N + 512 * (n + 1)],
                                    start=(r == 0 and p == 0),
                                    stop=(r == 3 and p == npass - 1),
                                    tile_position=(0, 32 * j),
                                )
                    # bank n complete: evacuate psum -> SBUF fp16, half on
                    # DVE, half on ACT
                    nc.vector.tensor_copy(evac[n][:, 0:256], psum[n][:, 0:256])
                    nc.scalar.copy(evac[n][:, 256:512], psum[n][:, 256:512])
                    if n == 2:
                        tail(0, last_main)
                        tail(1, last_main)
                tail(2, last_main)
                tail(3, last_main)

                hT = hT_new

            # ---- stage the final hT of timestep t to DRAM for host-side
            # readout (idle DMA engines; zero PE cost)
            def stage(t=t, hT=hT):
                for q in range(4):
                    eng = (nc.sync, nc.gpsimd)[q % 2]
                    eng.dma_start(
                        hs_d[t * 128 : (t + 1) * 128,
                             q * 4 * BPC : (q + 1) * 4 * BPC],
                        hT[q][:])

            pending.append(stage)

        for fn in pending:
            fn()
    nc.compile()
    return nc


def _prep_inputs(inputs, W_rec, W_in, b_in, sensory_indices, n_steps, mode=MODE):
    inputs = np.asarray(inputs, np.float32)
    W_rec = np.asarray(W_rec, np.float32)
    W_in = np.asarray(W_in, np.float32)
    b_in = np.asarray(b_in, np.float32)
    sens = np.asarray(sensory_indices).astype(np.int64)

    wtf = np.ascontiguousarray(W_rec.T)
    w1 = wtf.astype(np.float16)
    if mode == "fp16x2":
        w2 = (wtf - w1.astype(np.float32)).astype(np.float16)
        wt = np.concatenate([w1, w2], axis=0)
    else:
        wt = w1
    wt = np.ascontiguousarray(wt)
    i128 = (np.arange(128)[:, None] % 32 == np.arange(BPC)[None, :]).astype(np.float16)

    # dense injection in hT layout, per core: injd[t*128+m, c*8+b]
    inj_all = inputs[:, :n_steps, :] @ W_in.T + b_in  # [B, T, 256]
    inj_dense = np.zeros((B, n_steps, N), np.float32)
    np.add.at(inj_dense, (slice(None), slice(None), sens), inj_all)
    injd_cores = []
    for g in range(NCORES):
        a = inj_dense[g * BPC : (g + 1) * BPC]  # [8, T, 2048]
        a = a.reshape(BPC, n_steps, NCHUNK, 128).transpose(1, 3, 2, 0)
        injd_cores.append(np.ascontiguousarray(
            a.reshape(n_steps * 128, NCHUNK * BPC).astype(np.float16)))

    return wt, injd_cores, i128


def _run(inputs, W_rec, W_in, b_in, W_out, b_out, sensory_indices, output_indices,
         K, n_steps=T, trace=False, mode=MODE):
    from concourse.bass_utils import run_bass_kernel_spmd

    assert int(K) == 4
    wt, injd_cores, i128 = _prep_inputs(
        inputs, W_rec, W_in, b_in, sensory_indices, n_steps, mode)

    key = (n_steps, mode)
    if key not in _CACHE:
        _CACHE[key] = _build_nc(n_steps, mode)
    nc = _CACHE[key]

    in_maps = [
        {"wt": wt, "injd": injd_cores[g], "i128": i128}
        for g in range(NCORES)
    ]
    res = run_bass_kernel_spmd(nc, in_maps, list(range(NCORES)), trace=trace)

    # host-side readout: hs[t*128+m, q*32+c*8+b] -> h[t, b, (4q+c)*128+m]
    W_out = np.asarray(W_out, np.float32)
    b_out = np.asarray(b_out, np.float32)
    oidx = np.asarray(output_indices).astype(np.int64)
    wsel_full = np.zeros((N, 2), np.float32)
    np.add.at(wsel_full, oidx, W_out.T)
    outs = []
    for g in range(NCORES):
        hs = np.asarray(res.results[g]["hs"]).astype(np.float32)
        a = hs.reshape(n_steps, 128, 4, 4, BPC)  # [t, m, q, c, b]
        h = a.transpose(0, 4, 2, 3, 1).reshape(n_steps, BPC, N)  # [t, b, n]
        outs.append(np.einsum("tbn,no->bto", h, wsel_full))
    full = np.concatenate(outs, axis=0) + b_out  # [B, T, 2]
    return np.ascontiguousarray(full.astype(np.float32)), res


def kernel(**inputs):
    out, _ = _run(
        inputs["inputs"], inputs["W_rec"], inputs["W_in"], inputs["b_in"],
        inputs["W_out"], inputs["b_out"], inputs["sensory_indices"],
        inputs["output_indices"], inputs["K"],
    )
    return out


# revision 19
# speedup vs baseline: 1.0911x; 1.0911x over previous
"""Trainium2 Bass kernel for a dense recurrent scan (nn_CXBPU_55611236549128).

Math (per timestep t, K=4 microsteps):
    inj  = x_t @ W_in.T + b_in                  scattered into sensory_indices
    h    = relu(h @ W_rec.T + scatter(inj))     microstep 0
    h    = relu(h @ W_rec.T)                    microsteps 1..K-1
    out_t = h[:, output_indices] @ W_out.T + b_out

Sharding: data-parallel over batch, 8 rows per core, W_rec replicated.

Per-core design (feature-major "hT" layout [128 partitions, 16 chunks x 8 batch]):
  - W_rec.T resident in SBUF as fp16 (single pass; quantization noise averages
    out over the 2048-wide contraction, end-to-end rel err ~8e-4), streamed as
    the *moving* matmul operand every microstep. 4 PE column groups
    (tile_position=(0,32j)) give 4 concurrent 512-wide streams = the PE
    inflow roofline (~216ns per slot group of 4 MMs, 16 groups/microstep).
  - Every pipelined structure is split into per-bank / per-quarter TILES
    (4 psum banks, 4 evac tiles, 4 psumT tiles, 4 hT quarter tiles): Tile
    tracks dependencies at tile granularity, so a single wide tile creates
    false WAR/RAW edges that serialize the whole pipeline (this was the
    dominant cost of earlier versions).
  - Mains are bank-outer so each bank's evacuation (psum -> SBUF fp16, half
    on DVE, half on ACT) spreads across the stream instead of bunching at
    the end.
  - A "transpose-sum" matmul per 128-chunk against a 0/1 selector (i128)
    folds the 4 partition groups back into feature-major hT (exact in fp32
    PSUM). The 16 tMMs are pinned after the mains via add_dep_helper so the
    scheduler cannot head-of-line-block the PE queue with them. relu (DVE)
    per quarter; microstep 0 fuses the injection add (DMA'd dense, host
    precomputed) before the relu.
  - Readout is done ON THE HOST: the final hT of each timestep is staged to
    DRAM on otherwise-idle DMA engines (zero PE cost) and the tiny
    h[:, oidx] @ W_out.T runs in numpy after the kernel.
"""

import os
from contextlib import ExitStack

import numpy as np

N = 2048
B = 64
T = 128
NCORES = 8
BPC = B // NCORES  # 8 batch rows per core
NCHUNK = N // 128  # 16

_CACHE = {}

# 'fp16' = single-pass fp16 (fast), 'fp16x2' = two-pass fp16 split (more exact)
MODE = os.environ.get("KERNEL_MM_MODE", "fp16")


def _build_nc(n_steps, mode=MODE):
    import concourse.bass as bass
    import concourse.mybir as mybir
    import concourse.tile as tile
    from bass_rust import add_dep_helper
    from concourse import bacc

    f32 = mybir.dt.float32
    f16 = mybir.dt.float16
    fmm = f16
    npass = 2 if mode == "fp16x2" else 1
    nc = bacc.Bacc(trn_type="TRN2")

    NSLAB = npass * NCHUNK

    wt_d = nc.dram_tensor("wt", [NSLAB * 128, N], fmm, kind="ExternalInput")
    injd_d = nc.dram_tensor("injd", [n_steps * 128, NCHUNK * BPC], fmm,
                            kind="ExternalInput")
    i128_d = nc.dram_tensor("i128", [128, BPC], fmm, kind="ExternalInput")
    hs_d = nc.dram_tensor("hs", [n_steps * 128, NCHUNK * BPC], fmm,
                          kind="ExternalOutput")

    with tile.TileContext(nc) as tc, ExitStack() as ctx:
        const = ctx.enter_context(tc.tile_pool(name="const", bufs=1))
        hpool = ctx.enter_context(tc.tile_pool(name="h", bufs=2))
        ipool = ctx.enter_context(tc.tile_pool(name="injd", bufs=2))
        epool = ctx.enter_context(tc.tile_pool(name="evac", bufs=2))
        ppool = ctx.enter_context(tc.tile_pool(name="psum", bufs=1, space="PSUM"))
        tpool = ctx.enter_context(tc.tile_pool(name="psumT", bufs=1, space="PSUM"))

        # resident W^T slabs: slab u at cols [u*2048, ...). Spread the load
        # across both HWDGE families + SWDGE.
        wt = const.tile([128, NSLAB * N], fmm)
        for u in range(NSLAB):
            eng = (nc.sync, nc.scalar, nc.gpsimd)[u % 3]
            eng.dma_start(wt[:, u * N : (u + 1) * N], wt_d[u * 128 : (u + 1) * 128, :])
        i128 = const.tile([128, BPC], fmm)
        nc.sync.dma_start(i128[:], i128_d[:])

        # one PSUM tile per bank so evac reads of bank n don't create false
        # WAR edges against matmul writes of other banks (Tile tracks
        # dependencies at tile granularity)
        psum = [ppool.tile([128, 512], f32, name=f"psum{n}") for n in range(4)]
        for n in range(4):
            nc.vector.memset(psum[n][:], 0.0)

        # persistent transpose-sum targets
        psumT = [tpool.tile([128, 4 * BPC], f32, name=f"psumT{q}")
                 for q in range(4)]

        # hT split into 4 quarter tiles (chunks 4q..4q+3) so round-r matmuls
        # only depend on relu(r), not all four relus
        hT = [hpool.tile([128, 4 * BPC], fmm, name=f"hT{q}") for q in range(4)]
        for q in range(4):
            nc.vector.memset(hT[q][:], 0.0)

        tc.strict_bb_all_engine_barrier()

        # Work deferred into the next microstep's stream (previous timestep's
        # readout) so its PE waits land after the producing relu completes.
        pending = []

        for t in range(n_steps):
            injd = ipool.tile([128, NCHUNK * BPC], fmm)
            nc.sync.dma_start(injd[:], injd_d[t * 128 : (t + 1) * 128, :])
            for s in range(4):
                evac = [epool.tile([128, 512], fmm, name=f"evac{n}") for n in range(4)]
                hT_new = [hpool.tile([128, 4 * BPC], fmm, name=f"hTn{q}") for q in range(4)]

                if s == 0:
                    # previous timestep's h staging DMA (for host-side readout)
                    for fn in pending:
                        fn()
                    pending = []

                # ---- main rounds, bank-outer (banks complete early so their
                # evacuations spread across the whole stream instead of
                # bunching at the end)
                # Diagonal (bank, round) order: banks complete spread out
                # (groups 9/11/14/16) so their evacuations pipeline, while
                # round-3 stationaries are first consumed only at group 9, so
                # the previous microstep's evac3->tmm3->relu3 chain has ~2x
                # the slack it gets under plain bank-outer order.
                ORDER = [(0, 0), (1, 0), (2, 0), (0, 1), (1, 1), (0, 2),
                         (3, 0), (1, 2), (0, 3), (2, 1), (1, 3), (3, 1),
                         (2, 2), (2, 3), (3, 2), (3, 3)]
                last_main = None
                for p in range(npass):
                    for n, r in ORDER:
                        for j in range(4):
                            kk = 4 * r + j
                            u = p * NCHUNK + kk
                            last_main = nc.tensor.matmul(
                                psum[n][32 * j : 32 * j + BPC, :],
                                lhsT=hT[r][:, j * BPC : (j + 1) * BPC],
                                rhs=wt[:, u * N + 512 * n : u * N + 512 * (n + 1)],
                                start=(r == 0 and p == 0),
                                stop=(r == 3 and p == npass - 1),
                                tile_position=(0, 32 * j),
                            )
                        if r == 3 and p == npass - 1:
                            # bank n complete: evacuate psum -> SBUF fp16,
                            # half on DVE, half on ACT
                            nc.vector.tensor_copy(evac[n][:, 0:256],
                                                  psum[n][:, 0:256])
                            nc.scalar.copy(evac[n][:, 256:512],
                                           psum[n][:, 256:512])

                # ---- transpose-sum + relu, per chunk-quarter q (== psum bank
                # q == next microstep's round q). Pinned after the mains so
                # the scheduler cannot interleave them into the main stream
                # (head-of-line blocking on the in-order PE queue).
                prev_tmm = last_main
                for q in range(4):
                    for c in range(4):
                        mm = nc.tensor.matmul(
                            psumT[q][:, c * BPC : (c + 1) * BPC],
                            lhsT=evac[q][:, c * 128 : (c + 1) * 128],
                            rhs=i128[:],
                            start=True,
                            stop=True,
                        )
                        add_dep_helper(mm.ins, prev_tmm.ins, sync=False,
                                       reason="pin tmm after mains")
                        prev_tmm = mm
                    if s == 0:
                        # injection lands only on microstep 0: relu(psum + inj)
                        nc.vector.tensor_add(
                            hT_new[q][:], psumT[q][:],
                            injd[:, q * 4 * BPC : (q + 1) * 4 * BPC])
                        nc.vector.tensor_relu(hT_new[q][:], hT_new[q][:])
                    else:
                        nc.vector.tensor_relu(hT_new[q][:], psumT[q][:])

                hT = hT_new

            # ---- stage the final hT of timestep t to DRAM for host-side
            # readout (idle DMA engines; zero PE cost)
            def stage(t=t, hT=hT):
                for q in range(4):
                    eng = (nc.sync, nc.gpsimd)[q % 2]
                    eng.dma_start(
                        hs_d[t * 128 : (t + 1) * 128,
                             q * 4 * BPC : (q + 1) * 4 * BPC],
                        hT[q][:])

            pending.append(stage)

        for fn in pending:
            fn()
    nc.compile()
    return nc


def _prep_inputs(inputs, W_rec, W_in, b_in, sensory_indices, n_steps, mode=MODE):
    inputs = np.asarray(inputs, np.float32)
    W_rec = np.asarray(W_rec, np.float32)
    W_in = np.asarray(W_in, np.float32)
    b_in = np.asarray(b_in, np.float32)
    sens = np.asarray(sensory_indices).astype(np.int64)

    wtf = np.ascontiguousarray(W_rec.T)
    w1 = wtf.astype(np.float16)
    if mode == "fp16x2":
        w2 = (wtf - w1.astype(np.float32)).astype(np.float16)
        wt = np.concatenate([w1, w2], axis=0)
    else:
        wt = w1
    wt = np.ascontiguousarray(wt)
    i128 = (np.arange(128)[:, None] % 32 == np.arange(BPC)[None, :]).astype(np.float16)

    # dense injection in hT layout, per core: injd[t*128+m, c*8+b]
    inj_all = inputs[:, :n_steps, :] @ W_in.T + b_in  # [B, T, 256]
    inj_dense = np.zeros((B, n_steps, N), np.float32)
    np.add.at(inj_dense, (slice(None), slice(None), sens), inj_all)
    injd_cores = []
    for g in range(NCORES):
        a = inj_dense[g * BPC : (g + 1) * BPC]  # [8, T, 2048]
        a = a.reshape(BPC, n_steps, NCHUNK, 128).transpose(1, 3, 2, 0)
        injd_cores.append(np.ascontiguousarray(
            a.reshape(n_steps * 128, NCHUNK * BPC).astype(np.float16)))

    return wt, injd_cores, i128


def _run(inputs, W_rec, W_in, b_in, W_out, b_out, sensory_indices, output_indices,
         K, n_steps=T, trace=False, mode=MODE):
    from concourse.bass_utils import run_bass_kernel_spmd

    assert int(K) == 4
    wt, injd_cores, i128 = _prep_inputs(
        inputs, W_rec, W_in, b_in, sensory_indices, n_steps, mode)

    key = (n_steps, mode)
    if key not in _CACHE:
        _CACHE[key] = _build_nc(n_steps, mode)
    nc = _CACHE[key]

    in_maps = [
        {"wt": wt, "injd": injd_cores[g], "i128": i128}
        for g in range(NCORES)
    ]
    res = run_bass_kernel_spmd(nc, in_maps, list(range(NCORES)), trace=trace)

    # host-side readout: hs[t*128+m, q*32+c*8+b] -> h[t, b, (4q+c)*128+m]
    W_out = np.asarray(W_out, np.float32)
    b_out = np.asarray(b_out, np.float32)
    oidx = np.asarray(output_indices).astype(np.int64)
    wsel_full = np.zeros((N, 2), np.float32)
    np.add.at(wsel_full, oidx, W_out.T)
    outs = []
    for g in range(NCORES):
        hs = np.asarray(res.results[g]["hs"]).astype(np.float32)
        a = hs.reshape(n_steps, 128, 4, 4, BPC)  # [t, m, q, c, b]
        h = a.transpose(0, 4, 2, 3, 1).reshape(n_steps, BPC, N)  # [t, b, n]
        outs.append(np.einsum("tbn,no->bto", h, wsel_full))
    full = np.concatenate(outs, axis=0) + b_out  # [B, T, 2]
    return np.ascontiguousarray(full.astype(np.float32)), res


def kernel(**inputs):
    out, _ = _run(
        inputs["inputs"], inputs["W_rec"], inputs["W_in"], inputs["b_in"],
        inputs["W_out"], inputs["b_out"], inputs["sensory_indices"],
        inputs["output_indices"], inputs["K"],
    )
    return out


# revision 20
# speedup vs baseline: 1.1044x; 1.0122x over previous
"""Trainium2 Bass kernel for a dense recurrent scan (nn_CXBPU_55611236549128).

Math (per timestep t, K=4 microsteps):
    inj  = x_t @ W_in.T + b_in                  scattered into sensory_indices
    h    = relu(h @ W_rec.T + scatter(inj))     microstep 0
    h    = relu(h @ W_rec.T)                    microsteps 1..K-1
    out_t = h[:, output_indices] @ W_out.T + b_out

Sharding: data-parallel over batch, 8 rows per core, W_rec replicated.

Per-core design (feature-major "hT" layout [128 partitions, 16 chunks x 8 batch]):
  - W_rec.T resident in SBUF as fp16 (single pass; quantization noise averages
    out over the 2048-wide contraction, end-to-end rel err ~8e-4), streamed as
    the *moving* matmul operand every microstep. 4 PE column groups
    (tile_position=(0,32j)) give 4 concurrent 512-wide streams = the PE
    inflow roofline (~216ns per slot group of 4 MMs, 16 groups/microstep).
  - Every pipelined structure is split into per-bank / per-quarter TILES
    (4 psum banks, 4 evac tiles, 4 psumT tiles, 4 hT quarter tiles): Tile
    tracks dependencies at tile granularity, so a single wide tile creates
    false WAR/RAW edges that serialize the whole pipeline (this was the
    dominant cost of earlier versions).
  - Mains are bank-outer so each bank's evacuation (psum -> SBUF fp16, half
    on DVE, half on ACT) spreads across the stream instead of bunching at
    the end.
  - A "transpose-sum" matmul per 128-chunk against a 0/1 selector (i128)
    folds the 4 partition groups back into feature-major hT (exact in fp32
    PSUM). The 16 tMMs are pinned after the mains via add_dep_helper so the
    scheduler cannot head-of-line-block the PE queue with them. relu (DVE)
    per quarter; microstep 0 fuses the injection add (DMA'd dense, host
    precomputed) before the relu.
  - Readout is done ON THE HOST: the final hT of each timestep is staged to
    DRAM on otherwise-idle DMA engines (zero PE cost) and the tiny
    h[:, oidx] @ W_out.T runs in numpy after the kernel.
"""

import os
from contextlib import ExitStack

import numpy as np

N = 2048
B = 64
T = 128
NCORES = 8
BPC = B // NCORES  # 8 batch rows per core
NCHUNK = N // 128  # 16

_CACHE = {}

# 'fp16' = single-pass fp16 (fast), 'fp16x2' = two-pass fp16 split (more exact)
MODE = os.environ.get("KERNEL_MM_MODE", "fp16")


def _build_nc(n_steps, mode=MODE):
    import concourse.bass as bass
    import concourse.mybir as mybir
    import concourse.tile as tile
    from bass_rust import add_dep_helper
    from concourse import bacc

    f32 = mybir.dt.float32
    f16 = mybir.dt.float16
    fmm = f16
    npass = 2 if mode == "fp16x2" else 1
    nc = bacc.Bacc(trn_type="TRN2")

    NSLAB = npass * NCHUNK

    wt_d = nc.dram_tensor("wt", [NSLAB * 128, N], fmm, kind="ExternalInput")
    injd_d = nc.dram_tensor("injd", [n_steps * 128, NCHUNK * BPC], fmm,
                            kind="ExternalInput")
    i128_d = nc.dram_tensor("i128", [128, BPC], fmm, kind="ExternalInput")
    hs_d = nc.dram_tensor("hs", [n_steps * 128, NCHUNK * BPC], fmm,
                          kind="ExternalOutput")

    with tile.TileContext(nc) as tc, ExitStack() as ctx:
        const = ctx.enter_context(tc.tile_pool(name="const", bufs=1))
        hpool = ctx.enter_context(tc.tile_pool(name="h", bufs=2))
        ipool = ctx.enter_context(tc.tile_pool(name="injd", bufs=2))
        epool = ctx.enter_context(tc.tile_pool(name="evac", bufs=2))
        ppool = ctx.enter_context(tc.tile_pool(name="psum", bufs=1, space="PSUM"))
        tpool = ctx.enter_context(tc.tile_pool(name="psumT", bufs=1, space="PSUM"))

        # resident W^T slabs, one tile per slab so the first rounds of t=0
        # can start as soon as their own slabs arrive instead of waiting for
        # the whole 8.4MB load. Spread across both HWDGE families + SWDGE.
        wt = []
        for u in range(NSLAB):
            wtile = const.tile([128, N], fmm, name=f"wt{u}")
            eng = (nc.sync, nc.scalar, nc.gpsimd)[u % 3]
            eng.dma_start(wtile[:], wt_d[u * 128 : (u + 1) * 128, :])
            wt.append(wtile)
        i128 = const.tile([128, BPC], fmm)
        nc.sync.dma_start(i128[:], i128_d[:])

        # one PSUM tile per bank so evac reads of bank n don't create false
        # WAR edges against matmul writes of other banks (Tile tracks
        # dependencies at tile granularity)
        psum = [ppool.tile([128, 512], f32, name=f"psum{n}") for n in range(4)]
        for n in range(4):
            nc.vector.memset(psum[n][:], 0.0)

        # persistent transpose-sum targets
        psumT = [tpool.tile([128, 4 * BPC], f32, name=f"psumT{q}")
                 for q in range(4)]

        # hT split into 4 quarter tiles (chunks 4q..4q+3) so round-r matmuls
        # only depend on relu(r), not all four relus
        hT = [hpool.tile([128, 4 * BPC], fmm, name=f"hT{q}") for q in range(4)]
        for q in range(4):
            nc.vector.memset(hT[q][:], 0.0)

        # Work deferred into the next microstep's stream (previous timestep's
        # readout) so its PE waits land after the producing relu completes.
        pending = []

        for t in range(n_steps):
            injd = ipool.tile([128, NCHUNK * BPC], fmm)
            nc.sync.dma_start(injd[:], injd_d[t * 128 : (t + 1) * 128, :])
            for s in range(4):
                evac = [epool.tile([128, 512], fmm, name=f"evac{n}") for n in range(4)]
                hT_new = [hpool.tile([128, 4 * BPC], fmm, name=f"hTn{q}") for q in range(4)]

                if t == 0 and s == 0:
                    # h is zero at t=0: h1 = relu(inj), no matmuls needed
                    for q in range(4):
                        nc.vector.tensor_relu(
                            hT_new[q][:],
                            injd[:, q * 4 * BPC : (q + 1) * 4 * BPC])
                    hT = hT_new
                    continue

                if s == 0:
                    # previous timestep's h staging DMA (for host-side readout)
                    for fn in pending:
                        fn()
                    pending = []

                # ---- main rounds, bank-outer (banks complete early so their
                # evacuations spread across the whole stream instead of
                # bunching at the end)
                last_main = None
                for n in range(4):
                    for p in range(npass):
                        for r in range(4):
                            for j in range(4):
                                kk = 4 * r + j
                                u = p * NCHUNK + kk
                                last_main = nc.tensor.matmul(
                                    psum[n][32 * j : 32 * j + BPC, :],
                                    lhsT=hT[r][:, j * BPC : (j + 1) * BPC],
                                    rhs=wt[u][:, 512 * n : 512 * (n + 1)],
                                    start=(r == 0 and p == 0),
                                    stop=(r == 3 and p == npass - 1),
                                    tile_position=(0, 32 * j),
                                )
                    # bank n complete: evacuate psum -> SBUF fp16, half on
                    # DVE, half on ACT
                    nc.vector.tensor_copy(evac[n][:, 0:256], psum[n][:, 0:256])
                    nc.scalar.copy(evac[n][:, 256:512], psum[n][:, 256:512])

                # ---- transpose-sum + relu, per chunk-quarter q (== psum bank
                # q == next microstep's round q). Pinned after the mains so
                # the scheduler cannot interleave them into the main stream
                # (head-of-line blocking on the in-order PE queue).
                prev_tmm = last_main
                for q in range(4):
                    for c in range(4):
                        mm = nc.tensor.matmul(
                            psumT[q][:, c * BPC : (c + 1) * BPC],
                            lhsT=evac[q][:, c * 128 : (c + 1) * 128],
                            rhs=i128[:],
                            start=True,
                            stop=True,
                        )
                        add_dep_helper(mm.ins, prev_tmm.ins, sync=False,
                                       reason="pin tmm after mains")
                        prev_tmm = mm
                    if s == 0:
                        # injection lands only on microstep 0: relu(psum + inj)
                        nc.vector.tensor_add(
                            hT_new[q][:], psumT[q][:],
                            injd[:, q * 4 * BPC : (q + 1) * 4 * BPC])
                        nc.vector.tensor_relu(hT_new[q][:], hT_new[q][:])
                    else:
                        nc.vector.tensor_relu(hT_new[q][:], psumT[q][:])

                hT = hT_new

            # ---- stage the final hT of timestep t to DRAM for host-side
            # readout (idle DMA engines; zero PE cost)
            def stage(t=t, hT=hT):
                for q in range(4):
                    eng = (nc.sync, nc.gpsimd)[q % 2]
                    eng.dma_start(
                        hs_d[t * 128 : (t + 1) * 128,
                             q * 4 * BPC : (q + 1) * 4 * BPC],
                        hT[q][:])

            pending.append(stage)

        for fn in pending:
            fn()
    nc.compile()
    return nc


def _prep_inputs(inputs, W_rec, W_in, b_in, sensory_indices, n_steps, mode=MODE):
    inputs = np.asarray(inputs, np.float32)
    W_rec = np.asarray(W_rec, np.float32)
    W_in = np.asarray(W_in, np.float32)
    b_in = np.asarray(b_in, np.float32)
    sens = np.asarray(sensory_indices).astype(np.int64)

    wtf = np.ascontiguousarray(W_rec.T)
    w1 = wtf.astype(np.float16)
    if mode == "fp16x2":
        w2 = (wtf - w1.astype(np.float32)).astype(np.float16)
        wt = np.concatenate([w1, w2], axis=0)
    else:
        wt = w1
    wt = np.ascontiguousarray(wt)
    i128 = (np.arange(128)[:, None] % 32 == np.arange(BPC)[None, :]).astype(np.float16)

    # dense injection in hT layout, per core: injd[t*128+m, c*8+b]
    inj_all = inputs[:, :n_steps, :] @ W_in.T + b_in  # [B, T, 256]
    inj_dense = np.zeros((B, n_steps, N), np.float32)
    np.add.at(inj_dense, (slice(None), slice(None), sens), inj_all)
    injd_cores = []
    for g in range(NCORES):
        a = inj_dense[g * BPC : (g + 1) * BPC]  # [8, T, 2048]
        a = a.reshape(BPC, n_steps, NCHUNK, 128).transpose(1, 3, 2, 0)
        injd_cores.append(np.ascontiguousarray(
            a.reshape(n_steps * 128, NCHUNK * BPC).astype(np.float16)))

    return wt, injd_cores, i128


def _run(inputs, W_rec, W_in, b_in, W_out, b_out, sensory_indices, output_indices,
         K, n_steps=T, trace=False, mode=MODE):
    from concourse.bass_utils import run_bass_kernel_spmd

    assert int(K) == 4
    wt, injd_cores, i128 = _prep_inputs(
        inputs, W_rec, W_in, b_in, sensory_indices, n_steps, mode)

    key = (n_steps, mode)
    if key not in _CACHE:
        _CACHE[key] = _build_nc(n_steps, mode)
    nc = _CACHE[key]

    in_maps = [
        {"wt": wt, "injd": injd_cores[g], "i128": i128}
        for g in range(NCORES)
    ]
    res = run_bass_kernel_spmd(nc, in_maps, list(range(NCORES)), trace=trace)

    # host-side readout: hs[t*128+m, q*32+c*8+b] -> h[t, b, (4q+c)*128+m]
    W_out = np.asarray(W_out, np.float32)
    b_out = np.asarray(b_out, np.float32)
    oidx = np.asarray(output_indices).astype(np.int64)
    wsel_full = np.zeros((N, 2), np.float32)
    np.add.at(wsel_full, oidx, W_out.T)
    outs = []
    for g in range(NCORES):
        hs = np.asarray(res.results[g]["hs"]).astype(np.float32)
        a = hs.reshape(n_steps, 128, 4, 4, BPC)  # [t, m, q, c, b]
        h = a.transpose(0, 4, 2, 3, 1).reshape(n_steps, BPC, N)  # [t, b, n]
        outs.append(np.einsum("tbn,no->bto", h, wsel_full))
    full = np.concatenate(outs, axis=0) + b_out  # [B, T, 2]
    return np.ascontiguousarray(full.astype(np.float32)), res


def kernel(**inputs):
    out, _ = _run(
        inputs["inputs"], inputs["W_rec"], inputs["W_in"], inputs["b_in"],
        inputs["W_out"], inputs["b_out"], inputs["sensory_indices"],
        inputs["output_indices"], inputs["K"],
    )
    return out
